# revision 8
# baseline (speedup 1.0000x reference)
"""Trainium2 Bass kernel for BitLinear: y[b,s,o] = sum_d x[b,s,d] * w[o,d].

x: [4, 2048, 4096] f32, weight: [4096, 4096] int32 (values 0..255), y f32.

Strategy:
- Data-parallel over tokens: 8192 tokens -> 8 cores x 1024 tokens.
- Precision: SINGLE bf16 pass. Weight values 0..255 are exact in bf16
  (8-bit significand); only x is rounded to bf16. Measured on the actual
  (seeded) inputs this gives max-rel error ~1.9e-3 vs the 2e-2 gate --
  10x margin -- at HALF the tensor-engine work of a hi/lo split.
- W-stationary formulation: out yt[n, m] = W^T[k, n]^T @ X^T[k, m].
- DMA-instruction count is the binding constraint (~1.3 us per
  dma_start on a HWDGE ring, completion-serialized): the previous
  per-k-tile streaming (576 DMAs) capped the kernel at ~750 us. So:
  * W is HOST-PRE-TILED so each 256-output group's weights are ONE
    contiguous 2 MB DMA (16 per pass, sync ring), prefetched one
    group ahead.
  * X^T shard (8 MB bf16) arrives as 4 contiguous 2 MB DMAs (scalar
    ring) during the first group's compute and stays resident.
  * Each group's 4 PSUM banks are staged into one SBUF tile and
    written out as ONE fused 1 MB DMA (scalar ring).
- Host gathers per-core yt [4096, 1024] f32, transposes, concatenates.
"""

import sys

for _p in ("/opt/trn_rl_repo", "/root/.axon_site/_ro/trn_rl_repo"):
    if _p not in sys.path:
        sys.path.append(_p)

import numpy as np
import ml_dtypes

N_CORES = 8
TOKENS = 8192  # 4 * 2048
D_IN = 4096
D_OUT = 4096
T_SHARD = TOKENS // N_CORES  # 1024

_NC_CACHE = {}


def build_nc(repeats: int = 1):
    """Build (and cache) the Bass program.

    repeats > 1 re-emits the compute body (used only for slope-based HW
    timing; identical output)."""
    if repeats in _NC_CACHE:
        return _NC_CACHE[repeats]

    import concourse.mybir as mybir
    import concourse.tile as tile
    from concourse import bacc

    P = 128
    NG = D_OUT // 256   # 16 groups of 256 output features
    KT = D_IN // P      # 32 k-tiles
    XC = 4              # X chunks (8 k-tiles each)
    nc = bacc.Bacc(None, target_bir_lowering=False)
    with tile.TileContext(nc) as tc:
        with tc.tile_pool(name="dram", bufs=1, space="DRAM") as dram:
            kxm = dram.tile([D_IN, T_SHARD], mybir.dt.bfloat16,
                            kind="ExternalInput", name="kxm", uniquify=False)
            # pre-tiled W: [ng*128+p, kt*256+j] = W^T[kt*128+p, ng*256+j]
            kxns = dram.tile([NG * P, KT * 256], mybir.dt.bfloat16,
                             kind="ExternalInput", name="kxns", uniquify=False)
            yt = dram.tile([D_OUT, T_SHARD], mybir.dt.float32,
                           kind="ExternalOutput", name="yt", uniquify=False)
            # [128, 32, 1024] view of x^T: (kt, p) row-major in DRAM
            kxm3 = kxm[:].rearrange("(ko p) m -> p ko m", p=P)
            # [16, 128, 2, 1024] view of yt rows (ng, nsl, p)
            ytv = yt[:].rearrange("(g q p) m -> g p q m", q=2, p=P)
            with tc.tile_pool(name="xpool", bufs=XC) as xpool, \
                 tc.tile_pool(name="wpool", bufs=2) as wpool, \
                 tc.tile_pool(name="pspool", bufs=2, space="PSUM") as pspool, \
                 tc.tile_pool(name="evpool", bufs=2) as evpool:
                kc = KT // XC  # 8 k-tiles per chunk
                x3s = [None] * XC  # per-chunk [128, 8, 1024] views
                first = True
                wcur = None
                for rep in range(repeats):
                    for ng in range(NG):
                        if wcur is None:  # very first group
                            wcur = wpool.tile([P, KT * 256],
                                              mybir.dt.bfloat16,
                                              name="wt", tag="wt")
                            nc.sync.dma_start(
                                wcur[:], kxns[ng * P:(ng + 1) * P, :])
                        if first:
                            # X streams in as 4 contiguous 2MB chunks on
                            # the scalar ring, hidden under ng=0 compute.
                            for c in range(XC):
                                xt = xpool.tile([P, kc * T_SHARD],
                                                mybir.dt.bfloat16,
                                                name="xt", tag="xt")
                                x3 = xt[:].rearrange(
                                    "p (ko m) -> p ko m", ko=kc)
                                nc.scalar.dma_start(
                                    x3, kxm3[:, c * kc:(c + 1) * kc, :])
                                x3s[c] = x3
                            first = False
                        # prefetch next group's W ahead of this group's
                        # MMs (ring order: W(g+1) before evict(g))
                        ngn = (ng + 1) % NG
                        last = (rep == repeats - 1 and ng == NG - 1)
                        if not last:
                            wnext = wpool.tile([P, KT * 256],
                                               mybir.dt.bfloat16,
                                               name="wt", tag="wt")
                            nc.sync.dma_start(
                                wnext[:], kxns[ngn * P:(ngn + 1) * P, :])
                        b4 = pspool.tile([P, 2048], mybir.dt.float32,
                                         name="b4", tag="b4")
                        for k in range(KT):
                            for nsl in range(2):
                                lhsT = wcur[:, k * 256 + nsl * P:
                                            k * 256 + (nsl + 1) * P]
                                for mc in range(2):
                                    nc.tensor.matmul(
                                        b4[:, nsl * 1024 + mc * 512:
                                           nsl * 1024 + (mc + 1) * 512],
                                        lhsT,
                                        x3s[k // kc][:, k % kc,
                                                     mc * 512:(mc + 1) * 512],
                                        start=(k == 0),
                                        stop=(k == KT - 1),
                                    )
                        ev = evpool.tile([P, 2048], mybir.dt.float32,
                                         name="ev", tag="ev")
                        nc.vector.tensor_copy(out=ev[:], in_=b4[:])
                        ev3 = ev[:].rearrange("p (q m) -> p q m", q=2)
                        nc.scalar.dma_start(ytv[ng], ev3)
                        wcur = wnext if not last else None
    nc.compile()
    _NC_CACHE[repeats] = nc
    return nc


def prepare_in_maps(x: np.ndarray, weight: np.ndarray):
    """Host-side shard prep: single bf16 copy of x^T (per-core token
    slice); W^T pre-tiled so each 256-output group is contiguous."""
    bf16 = ml_dtypes.bfloat16
    x2 = np.ascontiguousarray(np.asarray(x).reshape(TOKENS, D_IN))
    kxm_full = np.ascontiguousarray(x2.astype(bf16).T)  # [D_IN, TOKENS]

    wt = np.asarray(weight).astype(np.float32).astype(bf16).T  # [k, n]
    # [kt, p, ng, j] -> [ng, p, kt, j]
    kxns = np.ascontiguousarray(
        wt.reshape(32, 128, 16, 256).transpose(2, 1, 0, 3)
    ).reshape(16 * 128, 32 * 256)

    in_maps = []
    for c in range(N_CORES):
        kxm_c = np.ascontiguousarray(
            kxm_full[:, c * T_SHARD:(c + 1) * T_SHARD])
        in_maps.append({"kxm": kxm_c, "kxns": kxns})
    return in_maps


def gather_output(results):
    y = np.concatenate(
        [np.ascontiguousarray(results[c]["yt"].T) for c in range(N_CORES)],
        axis=0)
    return y.reshape(4, 2048, D_OUT).astype(np.float32, copy=False)


def kernel(x: np.ndarray, weight: np.ndarray) -> np.ndarray:
    from concourse.bass_utils import run_bass_kernel_spmd

    nc = build_nc()
    in_maps = prepare_in_maps(x, weight)
    res = run_bass_kernel_spmd(nc, in_maps, core_ids=list(range(N_CORES)))
    return gather_output(res.results)
